# revision 9
# baseline (speedup 1.0000x reference)
"""Trainium2 Bass kernel for nn_BailingMoeBlock (8 NeuronCores).

Sharding:
  - rmsnorm1: token-parallel (each core norms its 128-token block), then
    AllGather of xT (bf16, feature-major).
  - Attention: tensor-parallel over heads (2 q-heads + 1 kv-head per core);
    dense proj row-parallel -> bf16 partial [T,h] summed via ReduceScatter
    (each core ends with its 128-token block of attn_out).
  - resid2 / rmsnorm2 / router: token-parallel on the owned block; AllGather
    x2T (bf16) + router combine weights (fp32).
  - MoE: expert-parallel (2 experts/core); shared expert column-parallel.
    Combined partial summed via two column-split ReduceScatters overlapped
    with the second half of the expert compute. Host concatenates blocks.

Heavy matmuls in bf16 (fp32 accumulate in PSUM); router in fp32.
"""
import sys
import types
import numpy as np

for _p in ("/opt/trn_rl_repo", "/opt/pypackages"):
    if _p not in sys.path:
        sys.path.append(_p)

import ml_dtypes  # noqa: E402
from concourse import bacc  # noqa: E402
import concourse.bass as bass  # noqa: E402
import concourse.tile as tile  # noqa: E402
import concourse.mybir as mybir  # noqa: E402
from concourse.bass_utils import run_bass_kernel_spmd  # noqa: E402
from concourse.masks import make_identity  # noqa: E402

F32 = mybir.dt.float32
BF16 = mybir.dt.bfloat16
BF16_NP = ml_dtypes.bfloat16

N_CORES = 8
T = 1024
H = 2048
NH = 16
NKV = 4
D = 128
E = 16
I_EXP = 512
SI = 1024
TOPK = 4
RMS_EPS = 1e-6
ROPE_THETA = 10000.0
Q_SIZE = NH * D          # 2048
KV_SIZE = NKV * D        # 512
TB = T // 128            # 8 token blocks
HC = H // 128            # 16 h chunks
NEG_BIG = -30000.0

RS1_DT = BF16
RS2_DT = BF16

X = mybir.AxisListType.X
ALU = mybir.AluOpType
ACTF = mybir.ActivationFunctionType


def build_nc(debug=False):
    nc = bacc.Bacc("TRN2", target_bir_lowering=False, debug=False,
                   num_devices=N_CORES)

    def din(name, shape, dt):
        return nc.dram_tensor(name, list(shape), dt, kind="ExternalInput").ap()

    def dout(name, shape, dt):
        return nc.dram_tensor(name, list(shape), dt, kind="ExternalOutput").ap()

    wqkv_d = din("wqkv", (128, HC, 512), BF16)
    wd_d = din("wdense", (128, 2, H), BF16)
    sw13_d = din("sw13", (128, HC, 256), BF16)
    sw2_d = din("sw2", (128, H), BF16)
    w13a_d = din("w13a", (8, 128, H), BF16)   # [cc][p][hc*128+ci]
    w13b_d = din("w13b", (8, 128, H), BF16)
    w2a_d = din("w2a", (128, 4, H), BF16)
    w2b_d = din("w2b", (128, 4, H), BF16)
    gate_d = din("gate", (128, HC, E), F32)
    hid_d = din("hid", (T, H), F32)
    res_d = din("res", (T, H), F32)
    hid_own_d = din("hid_own", (128, H), F32)
    res_own_d = din("res_own", (128, H), F32)
    ropeq_d = din("ropeq", (128, T), F32)   # rows 0:64 cos*s, 64:128 sin*s
    ropek_d = din("ropek", (128, T), F32)
    maskd_d = din("maskd", (128, 128), F32)
    sela_d = din("sela", (E, 128), F32)
    selb_d = din("selb", (E, 128), F32)

    out0_d = dout("out0", (128, H), F32)
    out1_d = dout("out1", (128, H), F32)
    dbg = {}
    if debug:
        dbg["q"] = dout("dbg_q", (128, T), F32)          # q head0 rotated
        dbg["att"] = dout("dbg_att", (128, H), F32)      # attn_out own block
        dbg["x2"] = dout("dbg_x2", (128, H), F32)        # x2 own block fp32
        dbg["comb"] = dout("dbg_comb", (128, E), F32)    # comb own block
        dbg["acta"] = dout("dbg_acta", (128, 4 * T), F32)  # act expert a

    with tile.TileContext(nc) as tc:
        with (
            tc.tile_pool(name="const", bufs=1) as pc,
            tc.tile_pool(name="weights", bufs=1) as pw,
            tc.tile_pool(name="big", bufs=1) as pbig,
            tc.tile_pool(name="stream", bufs=3) as pstream,
            tc.tile_pool(name="tmp", bufs=2) as ptmp,
            tc.tile_pool(name="psA", bufs=3, space="PSUM") as psA,
            tc.tile_pool(name="psB", bufs=2, space="PSUM") as psB,
            tc.tile_pool(name="dram", bufs=1, space="DRAM") as pd,
        ):
            # ---------------- constants ----------------
            ident_b = pc.tile([128, 128], BF16, name="ident_b")
            make_identity(nc, ident_b[:])
            ident_f = pc.tile([128, 128], F32, name="ident_f")
            make_identity(nc, ident_f[:])
            maskd = pc.tile([128, 128], F32, name="maskd")
            nc.sync.dma_start(maskd[:], maskd_d[:])
            ropeq = pc.tile([128, T], F32, name="ropeq")
            nc.sync.dma_start(ropeq[:], ropeq_d[:])
            ropek = pc.tile([128, T], F32, name="ropek")
            nc.sync.dma_start(ropek[:], ropek_d[:])
            gate_sb = pc.tile([128, HC, E], F32, name="gate_sb")
            nc.sync.dma_start(gate_sb[:], gate_d[:])
            sela_sb = pc.tile([E, 128], F32, name="sela_sb")
            nc.sync.dma_start(sela_sb[:], sela_d[:])
            selb_sb = pc.tile([E, 128], F32, name="selb_sb")
            nc.sync.dma_start(selb_sb[:], selb_d[:])

            # ---------------- DRAM internal buffers ----------------
            rs1_in = pd.tile([T, H], RS1_DT, name="rs1_in")
            a2a1_out = pd.tile([TB, 128, H], RS1_DT, name="a2a1_out")
            ag1_in = pd.tile([128, HC * 128], BF16, name="ag1_in")
            ag1_out = pd.tile([TB, 128, HC * 128], BF16, name="ag1_out",
                              addr_space="Shared")
            ag2_in = pd.tile([128, E], F32, name="ag2_in")
            ag2_out = pd.tile([TB, 128, E], F32, name="ag2_out",
                              addr_space="Shared")
            # final combine: column-quarter AllToAlls + local adds
            rs2_in = [pd.tile([T, H // 4], RS2_DT, name=f"rs2_in{i}")
                      for i in range(4)]
            a2a2_out = [pd.tile([TB, 128, H // 4], RS2_DT,
                                name=f"a2a2_out{i}")
                        for i in range(4)]

            rg = [list(range(N_CORES))]

            def rmsnorm_to(dst, resid_t, sq_t, also=None):
                """dst = resid_t * rsqrt(mean(resid_t^2)+eps); resid_t [128,H] f32.
                One fused DVE pass computes the scaled squares + row-sum."""
                ssum = ptmp.tile([128, 1], F32, name="ssum", tag="nrm")
                nc.scalar.activation(sq_t[:], resid_t[:], ACTF.Square,
                                     accum_out=ssum[:])
                var = ptmp.tile([128, 1], F32, name="var", tag="nrm")
                nc.vector.tensor_scalar(var[:], ssum[:], 1.0 / H, RMS_EPS,
                                        ALU.mult, ALU.add)
                sd = ptmp.tile([128, 1], F32, name="sd", tag="nrm")
                nc.scalar.activation(sd[:], var[:], ACTF.Sqrt)
                rstd = ptmp.tile([128, 1], F32, name="rstd", tag="nrm")
                nc.vector.reciprocal(rstd[:], sd[:])
                nc.vector.tensor_scalar_mul(dst[:], resid_t[:], rstd[:])
                if also is not None:
                    nc.vector.tensor_scalar_mul(also[:], resid_t[:], rstd[:])

            def transpose16_f32(src_f32, dst_f32, dst_bf):
                """16x [128,128] fp32 PE transposes of src [128, H];
                write fp32 into dst_f32 [128,HC,128] and bf16 into dst_bf."""
                for hg in range(8):
                    tp = psB.tile([128, 2, 128], F32, name=f"tpn_{hg}", tag="B")
                    for j in range(2):
                        hcc = hg * 2 + j
                        nc.tensor.transpose(
                            tp[:, j, :], src_f32[:, hcc * 128:(hcc + 1) * 128],
                            ident_f[:])
                    if dst_f32 is not None:
                        nc.vector.tensor_copy(
                            dst_f32[:, hg * 2:(hg + 1) * 2, :], tp[:])
                    if dst_bf is not None:
                        nc.vector.tensor_copy(
                            dst_bf[:, hg * 2:(hg + 1) * 2, :], tp[:])

            # ===== P0: resid + rmsnorm1 on all 8 blocks (replicated) =====
            resid_own = pbig.tile([128, H], F32, name="resid_own", tag="ro")
            xT = pbig.tile([128, HC, TB, 128], BF16, name="xT", tag="xT")
            for tci in range(TB):
                h_t = pstream.tile([128, H], F32, name=f"h_{tci}", tag="hr",
                                   bufs=3)
                nc.sync.dma_start(h_t[:], hid_d[tci * 128:(tci + 1) * 128, :])
                r_t = pstream.tile([128, H], F32, name=f"r_{tci}", tag="hr",
                                   bufs=3)
                nc.sync.dma_start(r_t[:], res_d[tci * 128:(tci + 1) * 128, :])
                nc.vector.tensor_tensor(h_t[:], h_t[:], r_t[:], ALU.add)
                x_t = ptmp.tile([128, H], BF16, name=f"x_{tci}", tag="x")
                rmsnorm_to(x_t, h_t, r_t)
                for hg in range(4):
                    tp = psB.tile([128, 4, 128], BF16, name=f"tp0_{hg}",
                                  tag="B")
                    for j in range(4):
                        hcc = hg * 4 + j
                        nc.tensor.transpose(
                            tp[:, j, :], x_t[:, hcc * 128:(hcc + 1) * 128],
                            ident_b[:])
                    nc.vector.tensor_copy(xT[:, hg * 4:(hg + 1) * 4, tci, :],
                                          tp[:])
            # own-block residual for P4 (hid_own+res_own inputs)
            ho_t = pstream.tile([128, H], F32, name="ho_t", tag="hr", bufs=3)
            nc.sync.dma_start(ho_t[:], hid_own_d[:])
            ro_t = pstream.tile([128, H], F32, name="ro_t", tag="hr", bufs=3)
            nc.sync.dma_start(ro_t[:], res_own_d[:])
            nc.vector.tensor_tensor(resid_own[:], ho_t[:], ro_t[:], ALU.add)

            # ============ P1: qkv + rope + v transpose ============
            wqkv_sb = pw.tile([128, HC, 512], BF16, name="wqkv_sb", tag="wA")
            nc.sync.dma_start(wqkv_sb[:], wqkv_d[:])

            q_sb = [None, None]
            k_sb = None
            v_tok = pbig.tile([128, TB, 128], BF16, name="v_tok", tag="v_tok")

            def rope_apply(ps, tbl, out_sb):
                t1 = ptmp.tile([128, T], F32, name="rope_t1", tag="rope1",
                               bufs=1)
                t2 = ptmp.tile([128, T], F32, name="rope_t2", tag="rope2",
                               bufs=1)
                nc.vector.tensor_tensor(t1[0:64, :], ps[0:64, :],
                                        tbl[0:64, :], ALU.mult)
                nc.vector.tensor_tensor(t1[64:128, :], ps[64:128, :],
                                        tbl[0:64, :], ALU.mult)
                nc.vector.tensor_tensor(t2[0:64, :], ps[64:128, :],
                                        tbl[64:128, :], ALU.mult)
                nc.vector.tensor_tensor(t2[64:128, :], ps[0:64, :],
                                        tbl[64:128, :], ALU.mult)
                nc.vector.tensor_tensor(out_sb[0:64, :], t1[0:64, :],
                                        t2[0:64, :], ALU.subtract)
                nc.vector.tensor_tensor(out_sb[64:128, :], t1[64:128, :],
                                        t2[64:128, :], ALU.add)

            for cc in range(4):
                ps = psA.tile([128, T], F32, name=f"qkv_{cc}", tag="A")
                for half in range(2):
                    for hc in range(HC):
                        nc.tensor.matmul(
                            ps[:, half * 512:(half + 1) * 512],
                            wqkv_sb[:, hc, cc * 128:(cc + 1) * 128],
                            xT[:, hc, half * 4:(half + 1) * 4, :],
                            start=(hc == 0), stop=(hc == HC - 1))
                if cc < 2:
                    q = pbig.tile([128, T], BF16, name=f"q_sb{cc}",
                                  tag=f"q{cc}")
                    rope_apply(ps, ropeq, q)
                    q_sb[cc] = q
                elif cc == 2:
                    k_sb = pbig.tile([128, T], BF16, name="k_sb", tag="k")
                    rope_apply(ps, ropek, k_sb)
                else:
                    v_bf = ptmp.tile([128, T], BF16, name="v_bf", tag="v_bf",
                                     bufs=1)
                    nc.vector.tensor_copy(v_bf[:], ps[:])
                    for tb2 in range(0, TB, 4):
                        tpv = psB.tile([128, 4, 128], BF16,
                                       name=f"tpv_{tb2}", tag="B")
                        for j in range(4):
                            nc.tensor.transpose(
                                tpv[:, j, :],
                                v_bf[:, (tb2 + j) * 128:(tb2 + j + 1) * 128],
                                ident_b[:])
                        nc.vector.tensor_copy(v_tok[:, tb2:tb2 + 4, :], tpv[:])

            if debug:
                nc.gpsimd.dma_start(dbg["q"][:], q_sb[0][:])

            # ============ P2+P3: attention + dense, grouped by 4 q-blocks ====
            wd_sb = pw.tile([128, 2, H], BF16, name="wd_sb", tag="wA")
            nc.sync.dma_start(wd_sb[:], wd_d[:])
            for grp in range(2):
                qlo = grp * 4              # q-blocks [qlo, qlo+4)
                nblk = qlo + 4             # tk blocks involved
                ctxg = [None, None]        # per head ctx psum [d, 4*128]
                probsTs = []
                for h in range(2):
                    pT = ptmp.tile([128, TB, 4, 128], BF16,
                                   name=f"pT_{h}_{grp}", tag=f"probsT{h}",
                                   bufs=1)
                    nc.vector.memset(pT[:, :nblk], 0.0)
                    probsTs.append(pT)
                for qj in range(4):
                    qi = qlo + qj
                    nk = (qi + 1) * 128
                    for h in range(2):
                        probsT = probsTs[h]
                        sc = psA.tile([128, T], F32, name=f"sc_{h}_{qi}",
                                      tag="A")
                        for b0 in range(0, nk, 512):
                            w = min(512, nk - b0)
                            nc.tensor.matmul(
                                sc[:, b0:b0 + w],
                                q_sb[h][:, qi * 128:(qi + 1) * 128],
                                k_sb[:, b0:b0 + w],
                                start=True, stop=True)
                        nc.vector.tensor_tensor(sc[:, qi * 128:nk],
                                                sc[:, qi * 128:nk],
                                                maskd[:], ALU.add)
                        negm = ptmp.tile([128, 1], F32, name="negm", tag="negm")
                        nc.vector.reduce_max(negm[:], sc[:, :nk], axis=X,
                                             negate=True)
                        ssum = ptmp.tile([128, 1], F32, name="ssum2",
                                         tag="negm")
                        pb = ptmp.tile([128, T], BF16, name=f"pb_{h}_{qi}",
                                       tag="pb")
                        nc.scalar.activation(pb[:, :nk], sc[:, :nk],
                                             ACTF.Exp, bias=negm[:],
                                             accum_out=ssum[:])
                        rinv = ptmp.tile([128, 1], F32, name="rinv", tag="negm")
                        nc.vector.reciprocal(rinv[:], ssum[:])
                        nc.vector.tensor_scalar_mul(pb[:, :nk], pb[:, :nk],
                                                    rinv[:])
                        for b0 in range(0, qi + 1, 4):
                            nb = min(4, qi + 1 - b0)
                            tpp = psB.tile([128, 4, 128], BF16,
                                           name=f"tpp_{b0}", tag="B")
                            for j in range(nb):
                                nc.tensor.transpose(
                                    tpp[:, j, :],
                                    pb[:, (b0 + j) * 128:(b0 + j + 1) * 128],
                                    ident_b[:])
                            nc.vector.tensor_copy(
                                probsT[:, b0:b0 + nb, qj, :], tpp[:, :nb, :])
                for h in range(2):
                    cps = psA.tile([128, 4 * 128], F32, name=f"ctx_{h}_{grp}",
                                   tag="A")
                    for b in range(nblk):
                        nc.tensor.matmul(cps[:], v_tok[:, b, :],
                                         probsTs[h][:, b, :, :],
                                         start=(b == 0), stop=(b == nblk - 1))
                    ctxg[h] = cps
                # dense for these 4 token blocks (token-major out)
                ctxb = ptmp.tile([128, 2, 4, 128], BF16, name=f"ctxb_{grp}",
                                 tag="ctxb", bufs=2)
                for h in range(2):
                    nc.vector.tensor_copy(ctxb[:, h, :, :], ctxg[h][:])
                for qj in range(4):
                    ti = qlo + qj
                    for g in range(4):
                        dps = psB.tile([128, 512], F32, name=f"d_{ti}_{g}",
                                       tag="B")
                        nc.tensor.matmul(dps[:], ctxb[:, 0, qj, :],
                                         wd_sb[:, 0, g * 512:(g + 1) * 512],
                                         start=True, stop=False)
                        nc.tensor.matmul(dps[:], ctxb[:, 1, qj, :],
                                         wd_sb[:, 1, g * 512:(g + 1) * 512],
                                         start=False, stop=True)
                        do = ptmp.tile([128, 512], RS1_DT, name=f"do_{ti}_{g}",
                                       tag="dout", bufs=3)
                        nc.vector.tensor_copy(do[:], dps[:])
                        nc.sync.dma_start(
                            rs1_in[ti * 128:(ti + 1) * 128,
                                   g * 512:(g + 1) * 512],
                            do[:])

            # ============ P4: A2A#1 + local 8-way add, resid2, rms2 ============
            nc.gpsimd.collective_compute(
                "AllToAll", ALU.bypass, replica_groups=rg,
                ins=[rs1_in.opt()], outs=[a2a1_out.opt()])

            # stage the 8 per-source partials for our token block, tree-add
            parts = pbig.tile([128, TB, H], RS1_DT, name="att_parts",
                              tag="xT")
            for hh in range(2):
                nc.sync.dma_start(
                    parts[:, hh * 4:(hh + 1) * 4, :],
                    a2a1_out[hh * 4:(hh + 1) * 4].rearrange("a p b -> p a b"))
            eng2 = (nc.vector, nc.gpsimd)
            sV = pstream.tile([128, H], F32, name="sV", tag="hr", bufs=3)
            sG = pstream.tile([128, H], F32, name="sG", tag="hr", bufs=3)
            nc.vector.tensor_tensor(sV[:], parts[:, 0, :], parts[:, 1, :],
                                    ALU.add)
            nc.gpsimd.tensor_tensor(sG[:], parts[:, 2, :], parts[:, 3, :],
                                    ALU.add)
            for i, (e, t) in enumerate(((nc.vector, sV), (nc.gpsimd, sG),
                                        (nc.vector, sV), (nc.gpsimd, sG))):
                e.tensor_tensor(t[:], t[:], parts[:, 4 + i, :], ALU.add)
            nc.vector.tensor_tensor(sV[:], sV[:], sG[:], ALU.add)
            resid2 = pstream.tile([128, H], F32, name="resid2", tag="hr",
                                  bufs=3)
            nc.gpsimd.tensor_tensor(resid2[:], sV[:], resid_own[:],
                                    ALU.add)
            nc.sync.dma_start(out1_d[:], resid2[:])
            if debug:
                nc.gpsimd.dma_start(dbg["att"][:], sV[:])

            sq_t4 = pstream.tile([128, H], F32, name="sq_t4", tag="hr", bufs=3)
            x2_f = pstream.tile([128, H], F32, name="x2_f", tag="hr", bufs=3)
            rmsnorm_to(x2_f, resid2, sq_t4)
            if debug:
                nc.sync.dma_start(dbg["x2"][:], x2_f[:])

            # transpose x2 -> bf16 AG input + fp32 router operand
            x2Tf = pbig.tile([128, HC, 128], F32, name="x2Tf", tag="x2Tf")
            x2T_own = ptmp.tile([128, HC, 128], BF16, name="x2T_own",
                                tag="xTown", bufs=1)
            transpose16_f32(x2_f, x2Tf, x2T_own)
            nc.sync.dma_start(ag1_in[:],
                              x2T_own[:].rearrange("p a b -> p (a b)"))
            nc.gpsimd.collective_compute(
                "AllGather", ALU.bypass, replica_groups=rg,
                ins=[ag1_in.opt()], outs=[ag1_out.opt()])

            # router in fp32 (after AG1 is issued; result feeds AG2 only)
            lg = psB.tile([128, E], F32, name="lg", tag="B")
            for hc in range(HC):
                nc.tensor.matmul(lg[:], x2Tf[:, hc, :], gate_sb[:, hc, :],
                                 start=(hc == 0), stop=(hc == HC - 1))
            negm1 = ptmp.tile([128, 1], F32, name="negm1", tag="negm")
            nc.vector.reduce_max(negm1[:], lg[:], axis=X, negate=True)
            ee = ptmp.tile([128, E], F32, name="ee", tag="t_ee", bufs=1)
            nc.scalar.activation(ee[:], lg[:], ACTF.Exp, bias=negm1[:])
            work = ptmp.tile([128, E], F32, name="work", tag="t_wk", bufs=1)
            nc.vector.tensor_copy(work[:], ee[:])
            mth = ptmp.tile([128, 1], F32, name="mth", tag="negm")
            nc.vector.reduce_max(mth[:], work[:], axis=X)
            msk = ptmp.tile([128, E], F32, name="msk", tag="t_mk", bufs=1)
            for _ in range(TOPK - 1):
                nc.vector.tensor_scalar(msk[:], work[:], mth[:], 1e30,
                                        ALU.is_ge, ALU.mult)
                nc.vector.tensor_tensor(work[:], work[:], msk[:], ALU.subtract)
                nc.vector.reduce_max(mth[:], work[:], axis=X)
            ge = ptmp.tile([128, E], F32, name="ge", tag="t_ge", bufs=1)
            nc.vector.tensor_scalar(ge[:], ee[:], mth[:], None, ALU.is_ge)
            cu = ptmp.tile([128, E], F32, name="cu", tag="t_cu", bufs=1)
            nc.vector.tensor_tensor(cu[:], ee[:], ge[:], ALU.mult)
            s4 = ptmp.tile([128, 1], F32, name="s4", tag="negm")
            nc.vector.reduce_sum(s4[:], cu[:], axis=X)
            ri4 = ptmp.tile([128, 1], F32, name="ri4", tag="negm")
            nc.vector.reciprocal(ri4[:], s4[:])
            comb = ptmp.tile([128, E], F32, name="comb", tag="t_cb", bufs=1)
            nc.vector.tensor_scalar_mul(comb[:], cu[:], ri4[:])
            if debug:
                nc.sync.dma_start(dbg["comb"][:], comb[:])
            nc.sync.dma_start(ag2_in[:], comb[:])
            nc.gpsimd.collective_compute(
                "AllGather", ALU.bypass, replica_groups=rg,
                ins=[ag2_in.opt()], outs=[ag2_out.opt()])

            # ============ P5: unpack x2T + combT ============
            x2T = pbig.tile([128, HC, TB, 128], BF16, name="x2T", tag="xT")
            for tb2 in range(TB):
                nc.sync.dma_start(
                    x2T[:, :, tb2, :],
                    ag1_out[tb2].rearrange("p (a b) -> p a b", a=HC))

            cp_sb = ptmp.tile([128, TB, E], F32, name="cp_sb", tag="cp",
                              bufs=1)
            nc.sync.dma_start(cp_sb[:], ag2_out.rearrange("a p e -> p a e"))
            combT = ptmp.tile([E, TB, 128], F32, name="combT", tag="combT",
                              bufs=1)
            for tb2 in range(TB):
                tpc = psB.tile([E, 128], F32, name=f"tpc_{tb2}", tag="B")
                nc.tensor.transpose(tpc[:], cp_sb[:, tb2, :], ident_f[:])
                nc.vector.tensor_copy(combT[:, tb2, :], tpc[:])

            # ============ P6: experts + shared ============
            sw13_sb = pw.tile([128, HC, 256], BF16, name="sw13_sb", tag="wA")
            nc.sync.dma_start(sw13_sb[:], sw13_d[:])
            sw2_sb = pw.tile([128, H], BF16, name="sw2_sb")
            nc.sync.dma_start(sw2_sb[:], sw2_d[:])

            act_sh = pbig.tile([128, TB, 128], BF16, name="act_sh",
                               tag="act_sh")
            gps_s = psA.tile([128, T], F32, name="gps_s", tag="A")
            ups_s = psA.tile([128, T], F32, name="ups_s", tag="A")
            for col, ps in ((0, gps_s), (1, ups_s)):
                for half in range(2):
                    for hc in range(HC):
                        nc.tensor.matmul(
                            ps[:, half * 512:(half + 1) * 512],
                            sw13_sb[:, hc, col * 128:(col + 1) * 128],
                            x2T[:, hc, half * 4:(half + 1) * 4, :],
                            start=(hc == 0), stop=(hc == HC - 1))
            sil_s = ptmp.tile([128, T], F32, name="sil_s", tag="sil", bufs=1)
            nc.scalar.activation(sil_s[:], gps_s[:], ACTF.Silu)
            nc.vector.tensor_tensor(
                act_sh[:].rearrange("p a b -> p (a b)"), sil_s[:], ups_s[:],
                ALU.mult)

            acts = []
            for ei, (w13_d_, sel_sb) in enumerate(
                    ((w13a_d, sela_sb), (w13b_d, selb_sb))):
                bps = psA.tile([128, T], F32, name=f"bps_{ei}", tag="A")
                for half in range(2):
                    nc.tensor.matmul(
                        bps[:, half * 512:(half + 1) * 512],
                        sel_sb[:], combT[:, half * 4:(half + 1) * 4, :],
                        start=True, stop=True)
                cb = ptmp.tile([128, T], F32, name=f"cb_{ei}", tag="cb",
                               bufs=1)
                nc.vector.tensor_copy(cb[:], bps[:])

                act_e = pbig.tile([128, 4, TB, 128], BF16, name=f"act_{ei}",
                                  tag=("x2Tf" if ei == 0 else "ro"))
                for cc in range(4):
                    wt_g = pstream.tile([128, HC, 128], BF16,
                                        name=f"wg_{ei}_{cc}", tag="w13")
                    nc.sync.dma_start(wt_g[:], w13_d_[cc].rearrange(
                        "p (a b) -> p a b", a=HC))
                    wt_u = pstream.tile([128, HC, 128], BF16,
                                        name=f"wu_{ei}_{cc}", tag="w13")
                    nc.sync.dma_start(wt_u[:], w13_d_[cc + 4].rearrange(
                        "p (a b) -> p a b", a=HC))
                    gps = psA.tile([128, T], F32, name=f"g_{ei}_{cc}", tag="A")
                    ups = psA.tile([128, T], F32, name=f"u_{ei}_{cc}", tag="A")
                    for half in range(2):
                        for hc in range(HC):
                            nc.tensor.matmul(
                                gps[:, half * 512:(half + 1) * 512],
                                wt_g[:, hc, :],
                                x2T[:, hc, half * 4:(half + 1) * 4, :],
                                start=(hc == 0), stop=(hc == HC - 1))
                    for half in range(2):
                        for hc in range(HC):
                            nc.tensor.matmul(
                                ups[:, half * 512:(half + 1) * 512],
                                wt_u[:, hc, :],
                                x2T[:, hc, half * 4:(half + 1) * 4, :],
                                start=(hc == 0), stop=(hc == HC - 1))
                    sil = ptmp.tile([128, T], F32, name=f"sil_{ei}_{cc}",
                                    tag="sil", bufs=1)
                    nc.scalar.activation(sil[:], gps[:], ACTF.Silu)
                    ut = ptmp.tile([128, T], F32, name=f"ut_{ei}_{cc}",
                                   tag="ut", bufs=1)
                    nc.vector.tensor_tensor(ut[:], ups[:], cb[:], ALU.mult)
                    nc.vector.tensor_tensor(
                        act_e[:, cc, :, :].rearrange("p a b -> p (a b)"),
                        sil[:], ut[:], ALU.mult)
                acts.append(act_e)

            if debug:
                nc.gpsimd.dma_start(
                    dbg["acta"][:], acts[0][:].rearrange("p a b c -> p (a b c)"))

            # w2 stage: token-major output; column-half i -> rs2_in[i]
            for g in range(4):
                w2g = []
                for ei, w2_d_ in enumerate((w2a_d, w2b_d)):
                    wt = pstream.tile([128, 4, 512], BF16,
                                      name=f"w2_{ei}_{g}", tag="w2g", bufs=3)
                    nc.sync.dma_start(wt[:],
                                      w2_d_[:, :, g * 512:(g + 1) * 512])
                    w2g.append(wt)
                for tb2 in range(TB):
                    ops = psB.tile([128, 512], F32, name=f"o_{g}_{tb2}",
                                   tag="B")
                    k = 0
                    for ei in range(2):
                        for ic in range(4):
                            nc.tensor.matmul(ops[:], acts[ei][:, ic, tb2, :],
                                             w2g[ei][:, ic, :],
                                             start=(k == 0), stop=False)
                            k += 1
                    nc.tensor.matmul(ops[:], act_sh[:, tb2, :],
                                     sw2_sb[:, g * 512:(g + 1) * 512],
                                     start=False, stop=True)
                    oo = ptmp.tile([128, 512], RS2_DT, name=f"oo_{g}_{tb2}",
                                   tag="dout", bufs=3)
                    nc.vector.tensor_copy(oo[:], ops[:])
                    nc.sync.dma_start(
                        rs2_in[g][tb2 * 128:(tb2 + 1) * 128, :],
                        oo[:])
                nc.gpsimd.collective_compute(
                    "AllToAll", ALU.bypass, replica_groups=rg,
                    ins=[rs2_in[g].opt()], outs=[a2a2_out[g].opt()])

            # local 8-way adds per column group, write f32 output directly
            for g in range(4):
                pg = ptmp.tile([128, TB, 512], RS2_DT, name=f"opart_{g}",
                               tag=f"probsT{g % 2}", bufs=1)
                nc.sync.dma_start(pg[:],
                                 a2a2_out[g].rearrange("a p b -> p a b"))
                oV = ptmp.tile([128, 512], F32, name=f"oV_{g}", tag="ctxb",
                               bufs=2)
                oG = ptmp.tile([128, 512], F32, name=f"oG_{g}", tag="pb",
                               bufs=2)
                nc.vector.tensor_tensor(oV[:], pg[:, 0, :], pg[:, 1, :],
                                        ALU.add)
                nc.gpsimd.tensor_tensor(oG[:], pg[:, 2, :], pg[:, 3, :],
                                        ALU.add)
                for i, (e, t) in enumerate(((nc.vector, oV), (nc.gpsimd, oG),
                                            (nc.vector, oV), (nc.gpsimd, oG))):
                    e.tensor_tensor(t[:], t[:], pg[:, 4 + i, :], ALU.add)
                og = ptmp.tile([128, 512], F32, name=f"og_{g}", tag="ogout",
                               bufs=2)
                nc.vector.tensor_tensor(og[:], oV[:], oG[:], ALU.add)
                nc.sync.dma_start(out0_d[:, g * 512:(g + 1) * 512], og[:])

    nc.compile()
    return nc


def prep_in_maps(inputs):
    """Shard/marshal full inputs into 8 per-core input maps."""
    f32 = np.float32
    hid = np.asarray(inputs["hidden_states"], f32)
    res = np.asarray(inputs["residual"], f32)
    rms1 = np.asarray(inputs["rms1_w"], f32)
    rms2 = np.asarray(inputs["rms2_w"], f32)
    w_qkv = np.asarray(inputs["w_qkv"], f32) * rms1[:, None]
    w_dense = np.asarray(inputs["w_dense"], f32)
    gate_w = np.asarray(inputs["gate_w"], f32) * rms2[:, None]
    w13 = np.asarray(inputs["w13"], f32) * rms2[None, :, None]
    w2 = np.asarray(inputs["w2"], f32)
    sw13 = np.asarray(inputs["sw13"], f32) * rms2[:, None]
    sw2 = np.asarray(inputs["sw2"], f32)
    pos = np.asarray(inputs["position_ids"]).astype(f32)

    inv_freq = (1.0 / (ROPE_THETA ** (np.arange(0, D, 2, dtype=f32) / D))).astype(f32)
    ang = pos[:, None] * inv_freq[None, :]          # [T, 64]
    cosT = np.cos(ang).T.astype(f32)                # [64, T]
    sinT = np.sin(ang).T.astype(f32)
    s = np.float32(D ** -0.5)
    ropeq = np.concatenate([cosT * s, sinT * s], 0)  # [128, T]
    ropek = np.concatenate([cosT, sinT], 0)

    ii = np.arange(128)
    maskd = np.where(ii[None, :] <= ii[:, None], 0.0, NEG_BIG).astype(f32)

    gate_dev = np.ascontiguousarray(
        gate_w.reshape(HC, 128, E).transpose(1, 0, 2))

    def bf(x):
        return np.ascontiguousarray(x.astype(BF16_NP))

    in_maps = []
    for c in range(N_CORES):
        kv = c // 2
        wq = w_qkv[:, 256 * c:256 * c + 256]
        wk = w_qkv[:, Q_SIZE + kv * 128:Q_SIZE + kv * 128 + 128]
        wv = w_qkv[:, Q_SIZE + KV_SIZE + kv * 128:Q_SIZE + KV_SIZE + kv * 128 + 128]
        wqkv_c = np.concatenate([wq, wk, wv], 1)          # [2048, 512]
        wqkv_dev = bf(wqkv_c.reshape(HC, 128, 512).transpose(1, 0, 2))

        wd_c = w_dense[256 * c:256 * c + 256, :]          # [256, 2048]
        wd_dev = bf(wd_c.reshape(2, 128, H).transpose(1, 0, 2))

        sw13_c = np.concatenate(
            [sw13[:, 128 * c:128 * c + 128],
             sw13[:, SI + 128 * c:SI + 128 * c + 128]], 1)  # [2048, 256]
        sw13_dev = bf(sw13_c.reshape(HC, 128, 256).transpose(1, 0, 2))
        sw2_dev = bf(sw2[128 * c:128 * c + 128, :])       # [128, 2048]

        def w13_dev(e):
            m = w13[e]                                    # [2048, 1024]
            return bf(m.reshape(HC, 128, 8, 128).transpose(2, 1, 0, 3)
                      .reshape(8, 128, H))

        def w2_dev(e):
            m = w2[e]                                     # [512, 2048]
            return bf(m.reshape(4, 128, H).transpose(1, 0, 2))

        sel = np.zeros((2, E, 128), f32)
        sel[0, 2 * c, :] = 1.0
        sel[1, 2 * c + 1, :] = 1.0

        in_maps.append({
            "hid": hid, "res": res,
            "wqkv": wqkv_dev, "wdense": wd_dev,
            "sw13": sw13_dev, "sw2": sw2_dev,
            "w13a": w13_dev(2 * c), "w13b": w13_dev(2 * c + 1),
            "w2a": w2_dev(2 * c), "w2b": w2_dev(2 * c + 1),
            "gate": gate_dev,
            "hid_own": np.ascontiguousarray(hid[128 * c:128 * c + 128]),
            "res_own": np.ascontiguousarray(res[128 * c:128 * c + 128]),
            "ropeq": ropeq, "ropek": ropek, "maskd": maskd,
            "sela": np.ascontiguousarray(sel[0]),
            "selb": np.ascontiguousarray(sel[1]),
        })
    return in_maps


_NC_CACHE = {}


def _get_nc(debug=False):
    key = debug
    if key not in _NC_CACHE:
        _NC_CACHE[key] = build_nc(debug=debug)
    return _NC_CACHE[key]


def run(inputs, debug=False, trace=False):
    nc = _get_nc(debug=debug)
    in_maps = prep_in_maps(inputs)
    kw = {}
    if trace:
        kw["trace"] = True
    res = run_bass_kernel_spmd(nc, in_maps, core_ids=list(range(N_CORES)), **kw)
    out0 = np.concatenate([res.results[c]["out0"] for c in range(N_CORES)], 0)
    out1 = np.concatenate([res.results[c]["out1"] for c in range(N_CORES)], 0)
    return (out0, out1), res


def kernel(**inputs):
    (out0, out1), _ = run(inputs)
    return out0, out1

